# revision 44
# baseline (speedup 1.0000x reference)
"""Trainium2 Bass kernel for nn_BiologicalSystemEncoder.

Data-parallel over 8 NeuronCores (batch 65536 -> 8192/core), 128-batch tiles,
two tiles software-pipelined through the emission order.

All linear layers are folded on the host into block-diagonal matrices
(W2+Wqkv -> M2, Wo+Wsys -> C, Wo_x -> F) so TensorE does every linear map:
  PE : x-transpose, x->h, h->qkv (stationary-activation trick), o->all_sys,
       all_sys->cross-qkv, oc->system_features, plus batch-major<->feature-major
       transposes.
The tiny per-batch attention cores run elementwise, bf16 for DVE 2x modes:
  DVE : score/attnV products + folds (systems 0-3), softmax plumbing, evictions
  GPS : score/attnV products (systems 4-7)
  ACT : exp, qkv PSUM evictions
"""
import numpy as np

B, N, NS, G, E, H = 65536, 64, 8, 8, 16, 4
Dh = E // H
NCORES = 8
BC = B // NCORES          # batch per core
NT = BC // 128            # tiles per core

_F32 = np.float32


# ---------------------------------------------------------------- host folds
def _fold_constants(inp):
    W1 = np.asarray(inp["W1"], np.float64)
    b1 = np.asarray(inp["b1"], np.float64)
    W2 = np.asarray(inp["W2"], np.float64)
    b2 = np.asarray(inp["b2"], np.float64)
    Wqkv = np.asarray(inp["Wqkv_sys"], np.float64)
    bqkv = np.asarray(inp["bqkv_sys"], np.float64)
    Wo = np.asarray(inp["Wo_sys"], np.float64)
    bo = np.asarray(inp["bo_sys"], np.float64)
    Wsys = np.asarray(inp["Wsys"], np.float64)
    bsys = np.asarray(inp["bsys"], np.float64)
    Wqkv_x = np.asarray(inp["Wqkv_x"], np.float64)
    bqkv_x = np.asarray(inp["bqkv_x"], np.float64)
    Wo_x = np.asarray(inp["Wo_x"], np.float64)
    bo_x = np.asarray(inp["bo_x"], np.float64)
    sys_idx = np.asarray(inp["sys_idx"], np.int64)

    scale = 1.0 / np.sqrt(Dh)

    # R1: [64, 1024]; chunk c columns c*128..: (g,e) -> relu-pre h feats
    # (biomarker order follows sys_idx so that system s == h-chunk s)
    R1 = np.zeros((64, 1024))
    b1c = np.zeros((128, 8))
    for c in range(8):
        for g in range(8):
            n = int(sys_idx[c, g])
            R1[n, c * 128 + g * 16:(c * 128) + (g + 1) * 16] = W1[n]
            b1c[g * 16:(g + 1) * 16, c] = b1[n]

    # M2: h-chunk(s) -> qkv_s. cols (pt, d, q, h): pt*128 + d*32 + q*4 + h
    M2 = np.zeros((8, 128, 384))
    bias2 = np.zeros((8, 384))
    for s in range(8):
        for g in range(8):
            n = int(sys_idx[s, g])
            A = np.einsum('ef,je->fj', W2[n], Wqkv[s])     # [f16, j48]
            bj = b2[n] @ Wqkv[s].T + bqkv[s]
            A[:, :E] *= scale
            bj[:E] *= scale
            for pt in range(3):
                for h in range(H):
                    for d in range(Dh):
                        j = pt * E + h * Dh + d
                        col = pt * 128 + d * 32 + g * 4 + h
                        M2[s, g * 16:(g + 1) * 16, col] = A[:, j]
                        bias2[s, col] = bj[j]

    # C: o-chunk(s) rows (d', q, h) -> sys_emb cols (s-block, e')
    C = np.zeros((128, 128))
    biasC = np.zeros(128)
    for s in range(8):
        Cfull = np.zeros((128, 16))          # rows (g,(h,d)) orig order
        for g in range(8):
            Cfull[g * 16:(g + 1) * 16, :] = (Wsys[s][:, g * 16:(g + 1) * 16] @ Wo[s]).T
        for g in range(8):
            for h in range(H):
                for d in range(Dh):
                    C[d * 32 + g * 4 + h, s * 16:(s + 1) * 16] = Cfull[g * 16 + h * Dh + d]
        biasC[s * 16:(s + 1) * 16] = Wsys[s] @ np.tile(bo[s], G) + bsys[s]

    # Mx: all_sys rows (s,e) -> cross qkv cols (pt, d, pos, h)
    Mx = np.zeros((128, 384))
    biasx = np.zeros(384)
    Aq = Wqkv_x.T.copy()
    Aq[:, :E] *= scale
    bx = bqkv_x.copy()
    bx[:E] *= scale
    for s in range(8):
        for pt in range(3):
            for h in range(H):
                for d in range(Dh):
                    j = pt * E + h * Dh + d
                    col = pt * 128 + d * 32 + s * 4 + h
                    Mx[s * 16:(s + 1) * 16, col] = Aq[:, j]
                    biasx[col] = bx[j]

    # F: oc rows (d', pos, h) -> sf cols (pos, e')
    F = np.zeros((128, 128))
    biasF = np.zeros(128)
    for s in range(8):
        for h in range(H):
            for d in range(Dh):
                F[d * 32 + s * 4 + h, s * 16:(s + 1) * 16] = Wo_x.T[h * Dh + d]
        biasF[s * 16:(s + 1) * 16] = bo_x

    assert abs(b1).max() == 0, "nonzero b1 unsupported (merged relu path)"
    assert abs(bias2).max() == 0 and abs(biasC).max() == 0, "nonzero folded biases unsupported"
    assert abs(biasx).max() == 0 and abs(biasF).max() == 0, "nonzero folded biases unsupported"

    return dict(
        R1m=R1.astype(_F32), b1m=b1c.astype(_F32),
        M2m=np.concatenate([M2[s] for s in range(8)], axis=1).astype(_F32),   # [128, 3072]
        Cm=C.astype(_F32), Mxm=Mx.astype(_F32), Fm=F.astype(_F32),
        Im=np.eye(128, dtype=_F32),
    )


# ---------------------------------------------------------------- bass build
def _build_bass(ntiles):
    import concourse.bass as bass
    import concourse.bacc as bacc
    import concourse.mybir as mybir
    from concourse import tile
    from concourse.mybir import AluOpType as Op, ActivationFunctionType as Act

    f32 = mybir.dt.float32
    nc = bacc.Bacc(trn_type="TRN2")

    xd = nc.dram_tensor("xc", [ntiles * 128, 64], f32, kind="ExternalInput")
    R1d = nc.dram_tensor("R1m", [64, 1024], f32, kind="ExternalInput")
    b1d = nc.dram_tensor("b1m", [128, 8], f32, kind="ExternalInput")
    M2d = nc.dram_tensor("M2m", [128, 3072], f32, kind="ExternalInput")
    Cd = nc.dram_tensor("Cm", [128, 128], f32, kind="ExternalInput")
    Mxd = nc.dram_tensor("Mxm", [128, 384], f32, kind="ExternalInput")
    Fd = nc.dram_tensor("Fm", [128, 128], f32, kind="ExternalInput")
    Id = nc.dram_tensor("Im", [128, 128], f32, kind="ExternalInput")
    sfd = nc.dram_tensor("sf", [ntiles * 128, 128], f32, kind="ExternalOutput")
    asd = nc.dram_tensor("asys", [ntiles * 128, 128], f32, kind="ExternalOutput")

    bf = mybir.dt.bfloat16
    with tile.TileContext(nc) as tc:
        cp = tc.alloc_tile_pool(name="consts", bufs=1)
        R1s = cp.tile([64, 1024], f32)
        nc.sync.dma_start(R1s[:], R1d[:])
        b1s = cp.tile([128, 8], f32)
        nc.sync.dma_start(b1s[:], b1d[:])
        M2s = cp.tile([128, 3072], f32)
        nc.sync.dma_start(M2s[:], M2d[:])
        Cs = cp.tile([128, 128], f32)
        nc.sync.dma_start(Cs[:], Cd[:])
        Mxs = cp.tile([128, 384], f32)
        nc.sync.dma_start(Mxs[:], Mxd[:])
        Fs = cp.tile([128, 128], f32)
        nc.sync.dma_start(Fs[:], Fd[:])
        Is = cp.tile([128, 128], f32)
        nc.sync.dma_start(Is[:], Id[:])

        xp = tc.alloc_tile_pool(name="xin", bufs=3)
        pst = tc.alloc_tile_pool(name="pst", bufs=2, space="PSUM")
        psh = tc.alloc_tile_pool(name="psh", bufs=2, space="PSUM")
        psq = tc.alloc_tile_pool(name="psq", bufs=2, space="PSUM")
        psm = tc.alloc_tile_pool(name="psm", bufs=2, space="PSUM")
        hp = tc.alloc_tile_pool(name="hp", bufs=3)
        qkvp = tc.alloc_tile_pool(name="qkvp", bufs=3)
        bigp = tc.alloc_tile_pool(name="bigp", bufs=3)
        s2kp = tc.alloc_tile_pool(name="s2kp", bufs=3)
        denp = tc.alloc_tile_pool(name="denp", bufs=3)
        op_ = tc.alloc_tile_pool(name="op", bufs=2)
        tinyp = tc.alloc_tile_pool(name="tinyp", bufs=8)
        otp = tc.alloc_tile_pool(name="otp", bufs=2)
        outp = tc.alloc_tile_pool(name="outp", bufs=3)

        for t in range(ntiles):
            rows = slice(t * 128, (t + 1) * 128)
            # ---- x load + transpose
            xt = xp.tile([128, 64], f32, tag="xt")
            nc.sync.dma_start(xt[:], xd[rows, :])
            xT_ps = pst.tile([64, 128], f32, tag="tps")
            nc.tensor.transpose(xT_ps[:], xt[:], Is[:])
            xTs = xp.tile([64, 128], f32, tag="xTs")
            nc.vector.tensor_copy(xTs[:], xT_ps[:])

            # ---- h chunks (feature-major, fp32); 4 chunks share a PSUM bank
            # (b1 == 0 asserted at fold time, so relu needs no per-chunk bias)
            hgrp = []
            for g2 in range(2):
                hps = psh.tile([128, 512], f32, tag="hps")
                for c4 in range(4):
                    c = g2 * 4 + c4
                    nc.tensor.matmul(hps[:, c4 * 128:(c4 + 1) * 128],
                                     R1s[:, c * 128:(c + 1) * 128], xTs[:])
                hg = hp.tile([128, 512], f32, tag=f"hg{g2}")
                nc.vector.tensor_scalar(hg[:], hps[:], 0.0, None, Op.max)
                hgrp.append(hg)
            hs = [hgrp[c // 4][:, (c % 4) * 128:(c % 4 + 1) * 128] for c in range(8)]

            # ---- qkv (batch-major; evicted to bf16 by ACT)
            # QKV layout (pt,s,d,q,h): addr = pt*1024 + s*128 + d*32 + q*4 + h
            QKV = qkvp.tile([128, 3072], bf, tag="qkv")
            QKVv = QKV[:].rearrange("p (pt s d q h) -> p pt s d q h", pt=3, s=8, d=4, q=8, h=4)
            for s in range(8):
                qps = psq.tile([128, 384], f32, tag="qps")
                nc.tensor.matmul(qps[:], hs[s], M2s[:, s * 384:(s + 1) * 384])
                dst = QKVv[:, :, s]
                src = qps[:].rearrange("p (pt d q h) -> p pt d q h", pt=3, d=4, q=8, h=4)
                nc.scalar.copy(dst, src)

            Qv = QKVv[:, 0]    # [128, s8, d4, q8, h4]
            Kv = QKVv[:, 1]
            Vv = QKVv[:, 2]
            Qb = Qv.unsqueeze(4).broadcast_to([128, 8, 4, 8, 8, 4])   # (s,d,q,k0,h)
            Kb = Kv.unsqueeze(3).broadcast_to([128, 8, 4, 8, 8, 4])   # (s,d,q0,k,h)
            Vb = Vv.unsqueeze(3).broadcast_to([128, 8, 4, 8, 8, 4])

            # ---- system attention (bf16 core; exp on ACT)
            # T layout (s,d,q,k,h); systems 0-3 on DVE, 4-7 on GPSIMD
            T = bigp.tile([128, 8192], bf, tag="big")
            T6 = T[:].rearrange("p (s d q k h) -> p s d q k h", s=8, d=4, q=8, k=8, h=4)
            nc.vector.tensor_tensor(T6[:, 0:4], Qb[:, 0:4], Kb[:, 0:4], Op.mult)
            nc.gpsimd.tensor_tensor(T6[:, 4:8], Qb[:, 4:8], Kb[:, 4:8], Op.mult)
            S2 = s2kp.tile([128, 4096], bf, tag="s4k")
            S2v = S2[:].rearrange("p (s d q k h) -> p s d q k h", s=8, d=2, q=8, k=8, h=4)
            nc.vector.tensor_tensor(S2v[:], T6[:, :, 0:2], T6[:, :, 2:4], Op.add)
            S = s2kp.tile([128, 2048], bf, tag="s2k")
            Sv = S[:].rearrange("p (s q k h) -> p s q k h", s=8, q=8, k=8, h=4)
            nc.vector.tensor_tensor(Sv[:].unsqueeze(2), S2v[:, :, 0:1], S2v[:, :, 1:2], Op.add)
            Ee = s2kp.tile([128, 2048], bf, tag="s2k")
            nc.scalar.activation(Ee[:, 0:1024], S[:, 0:1024], Act.Exp)
            nc.scalar.activation(Ee[:, 1024:2048], S[:, 1024:2048], Act.Exp)
            # denominators: fold over k; layout (s,q,k,h)
            Ev = Ee[:].rearrange("p (sq k h) -> p sq k h", sq=64, k=8, h=4)
            D2 = denp.tile([128, 64, 4, 4], f32, tag="d2")
            nc.vector.tensor_tensor(D2[:], Ev[:, :, 0:4], Ev[:, :, 4:8], Op.add)
            D4 = denp.tile([128, 64, 2, 4], f32, tag="d4")
            nc.vector.tensor_tensor(D4[:], D2[:, :, 0:2], D2[:, :, 2:4], Op.add)
            Dd = denp.tile([128, 64, 1, 4], f32, tag="dd")
            nc.vector.tensor_tensor(Dd[:], D4[:, :, 0:1], D4[:, :, 1:2], Op.add)
            Rv = denp.tile([128, 256], f32, tag="rv")
            nc.vector.reciprocal(Rv[:], Dd[:].rearrange("p a b c -> p (a b c)"))
            # attn @ V: P layout (s,d,q,k,h)
            Eb = Ee[:].rearrange("p (s q k h) -> p s q k h", s=8, q=8, k=8, h=4)
            Eb = Eb.unsqueeze(2).broadcast_to([128, 8, 4, 8, 8, 4])   # (s,d0,q,k,h)
            P = bigp.tile([128, 8192], bf, tag="big")
            P6 = P[:].rearrange("p (s d q k h) -> p s d q k h", s=8, d=4, q=8, k=8, h=4)
            nc.vector.tensor_tensor(P6[:, 0:4], Eb[:, 0:4], Vb[:, 0:4], Op.mult)
            nc.gpsimd.tensor_tensor(P6[:, 4:8], Eb[:, 4:8], Vb[:, 4:8], Op.mult)
            Pv = P[:].rearrange("p (sdq k h) -> p sdq k h", sdq=256, k=8, h=4)
            K1 = s2kp.tile([128, 256, 4, 4], bf, tag="s4k")
            nc.vector.tensor_tensor(K1[:, 0:128], Pv[:, 0:128, 0:4], Pv[:, 0:128, 4:8], Op.add)
            nc.vector.tensor_tensor(K1[:, 128:256], Pv[:, 128:256, 0:4], Pv[:, 128:256, 4:8], Op.add)
            K2 = s2kp.tile([128, 256, 2, 4], bf, tag="s2k")
            nc.vector.tensor_tensor(K2[:], K1[:, :, 0:2], K1[:, :, 2:4], Op.add)
            num = op_.tile([128, 256, 1, 4], bf, tag="num")
            nc.vector.tensor_tensor(num[:], K2[:, :, 0:1], K2[:, :, 1:2], Op.add)
            o = op_.tile([128, 1024], f32, tag="o")
            # o layout (s, d, q, h): system chunks are contiguous 128-col slices
            ov = o[:].rearrange("p (s d q h) -> p s d q h", s=8, d=4, q=8, h=4)
            nc.vector.tensor_tensor(
                ov[:],
                num[:].rearrange("p a b c -> p (a b c)").rearrange("p (s d q h) -> p s d q h", s=8, d=4, q=8, h=4),
                Rv[:].rearrange("p (s q h) -> p s q h", s=8, q=8, h=4).unsqueeze(2).broadcast_to([128, 8, 4, 8, 4]),
                Op.mult)

            # ---- o -> all_sys (4 transposes share a PSUM bank; 2 evicts)
            as_ps = psm.tile([128, 128], f32, tag="mps")
            oTg = []
            for g2 in range(2):
                oT_ps = pst.tile([128, 512], f32, tag="tps")
                for s4 in range(4):
                    s = g2 * 4 + s4
                    nc.tensor.transpose(oT_ps[:, s4 * 128:(s4 + 1) * 128],
                                        o[:, s * 128:(s + 1) * 128], Is[:])
                og = otp.tile([128, 512], f32, tag=f"oTg{g2}")
                nc.vector.tensor_copy(og[:], oT_ps[:])
                oTg.append(og)
            for s in range(8):
                nc.tensor.matmul(as_ps[:, s * 16:(s + 1) * 16],
                                 oTg[s // 4][:, (s % 4) * 128:(s % 4 + 1) * 128],
                                 Cs[:, s * 16:(s + 1) * 16])
            asys_s = outp.tile([128, 128], f32, tag="asys")
            nc.vector.tensor_copy(asys_s[:], as_ps[:])
            nc.sync.dma_start(asd[rows, :], asys_s[:])

            # ---- cross qkv
            asT_ps = pst.tile([128, 128], f32, tag="tps")
            nc.tensor.transpose(asT_ps[:], asys_s[:], Is[:])
            asTs = otp.tile([128, 128], f32, tag="asTs")
            nc.vector.tensor_copy(asTs[:], asT_ps[:])
            cq_ps = psq.tile([128, 384], f32, tag="qps")
            nc.tensor.matmul(cq_ps[:], asTs[:], Mxs[:])
            QKVc = qkvp.tile([128, 384], bf, tag="qkvc")
            nc.vector.tensor_copy(QKVc[:], cq_ps[:])
            QKVcv = QKVc[:].rearrange("p (pt d s h) -> p pt d s h", pt=3, d=4, s=8, h=4)
            Qcb = QKVcv[:, 0].unsqueeze(3).broadcast_to([128, 4, 8, 8, 4])
            Kcb = QKVcv[:, 1].unsqueeze(2).broadcast_to([128, 4, 8, 8, 4])
            Vcb = QKVcv[:, 2].unsqueeze(2).broadcast_to([128, 4, 8, 8, 4])

            # ---- cross attention (bf16 core)
            Tc = op_.tile([128, 1024], bf, tag="tc")
            Tc5 = Tc[:].rearrange("p (d q k h) -> p d q k h", d=4, q=8, k=8, h=4)
            nc.vector.tensor_tensor(Tc5, Qcb, Kcb, Op.mult)
            Sc2 = tinyp.tile([128, 512], bf, tag="tiny")
            nc.vector.tensor_tensor(Sc2[:], Tc[:, 0:512], Tc[:, 512:1024], Op.add)
            Sc = tinyp.tile([128, 256], bf, tag="tiny")
            nc.vector.tensor_tensor(Sc[:], Sc2[:, 0:256], Sc2[:, 256:512], Op.add)
            Ec = tinyp.tile([128, 256], bf, tag="tiny")
            nc.scalar.activation(Ec[:], Sc[:], Act.Exp)
            Ecv = Ec[:].rearrange("p (q k h) -> p q k h", q=8, k=8, h=4)
            Dc2 = tinyp.tile([128, 8, 4, 4], f32, tag="tiny")
            nc.vector.tensor_tensor(Dc2[:], Ecv[:, :, 0:4], Ecv[:, :, 4:8], Op.add)
            Dc4 = tinyp.tile([128, 8, 2, 4], f32, tag="tiny")
            nc.vector.tensor_tensor(Dc4[:], Dc2[:, :, 0:2], Dc2[:, :, 2:4], Op.add)
            Dc = tinyp.tile([128, 8, 1, 4], f32, tag="tiny")
            nc.vector.tensor_tensor(Dc[:], Dc4[:, :, 0:1], Dc4[:, :, 1:2], Op.add)
            Rc = tinyp.tile([128, 32], f32, tag="tinyf")
            nc.vector.reciprocal(Rc[:], Dc[:].rearrange("p a b c -> p (a b c)"))
            Ecb = Ecv.unsqueeze(1).broadcast_to([128, 4, 8, 8, 4])
            Pc = op_.tile([128, 1024], bf, tag="pc")
            Pc5 = Pc[:].rearrange("p (d q k h) -> p d q k h", d=4, q=8, k=8, h=4)
            nc.vector.tensor_tensor(Pc5, Ecb, Vcb, Op.mult)
            Pcv = Pc[:].rearrange("p (dq k h) -> p dq k h", dq=32, k=8, h=4)
            Kc1 = tinyp.tile([128, 32, 4, 4], bf, tag="tiny")
            nc.vector.tensor_tensor(Kc1[:], Pcv[:, :, 0:4], Pcv[:, :, 4:8], Op.add)
            Kc2 = tinyp.tile([128, 32, 2, 4], bf, tag="tiny")
            nc.vector.tensor_tensor(Kc2[:], Kc1[:, :, 0:2], Kc1[:, :, 2:4], Op.add)
            numc = tinyp.tile([128, 32, 1, 4], bf, tag="tiny")
            nc.vector.tensor_tensor(numc[:], Kc2[:, :, 0:1], Kc2[:, :, 1:2], Op.add)
            oc = tinyp.tile([128, 128], f32, tag="tinyf")
            nc.vector.tensor_tensor(
                oc[:].rearrange("p (dp r) -> p dp r", dp=4, r=32),
                numc[:].rearrange("p a b c -> p (a b c)").rearrange("p (dp r) -> p dp r", dp=4, r=32),
                Rc[:].unsqueeze(1).broadcast_to([128, 4, 32]), Op.mult)

            # ---- oc -> system_features
            ocT_ps = pst.tile([128, 128], f32, tag="tps")
            nc.tensor.transpose(ocT_ps[:], oc[:], Is[:])
            ocTs = otp.tile([128, 128], f32, tag="ocTs")
            nc.vector.tensor_copy(ocTs[:], ocT_ps[:])
            sf_ps = psm.tile([128, 128], f32, tag="mps")
            nc.tensor.matmul(sf_ps[:], ocTs[:], Fs[:])
            sf_s = outp.tile([128, 128], f32, tag="sf")
            nc.vector.tensor_copy(sf_s[:], sf_ps[:])
            nc.sync.dma_start(sfd[rows, :], sf_s[:])

        for pool in (outp, otp, tinyp, op_, denp, s2kp, bigp, qkvp, hp, psm, psq, psh, pst, xp, cp):
            pool.release()

    nc.finalize()
    return nc


_CACHE = {}
LAST_RESULT = None


def _get_nc(ntiles):
    if ntiles not in _CACHE:
        _CACHE[ntiles] = _build_bass(ntiles)
    return _CACHE[ntiles]


# ---------------------------------------------------------------- entry point
def kernel(**inputs):
    import os
    os.environ.setdefault("BASS_NEVER_TRACE", "1")  # no NTFF hook in this axon client
    from concourse.bass_utils import run_bass_kernel_spmd

    x = np.ascontiguousarray(np.asarray(inputs["x"], _F32))
    Bn = x.shape[0]
    consts = _fold_constants(inputs)

    ncores = NCORES
    bc = Bn // ncores
    ntiles = bc // 128
    nc = _get_nc(ntiles)

    in_maps = []
    for c in range(ncores):
        m = {"xc": x[c * bc:(c + 1) * bc]}
        m.update(consts)
        in_maps.append(m)

    res = run_bass_kernel_spmd(nc, in_maps, core_ids=list(range(ncores)))
    global LAST_RESULT
    LAST_RESULT = res
    sf = np.concatenate([res.results[c]["sf"] for c in range(ncores)], axis=0)
    asys = np.concatenate([res.results[c]["asys"] for c in range(ncores)], axis=0)
    return sf, asys.reshape(Bn, NS, E)


# revision 45
# speedup vs baseline: 1.0059x; 1.0059x over previous
"""Trainium2 Bass kernel for nn_BiologicalSystemEncoder.

Data-parallel over 8 NeuronCores (batch 65536 -> 8192/core), 128-batch tiles,
two tiles software-pipelined through the emission order.

All linear layers are folded on the host into block-diagonal matrices
(W2+Wqkv -> M2, Wo+Wsys -> C, Wo_x -> F) so TensorE does every linear map:
  PE : x-transpose, x->h, h->qkv (stationary-activation trick), o->all_sys,
       all_sys->cross-qkv, oc->system_features, plus batch-major<->feature-major
       transposes.
The tiny per-batch attention cores run elementwise, bf16 for DVE 2x modes:
  DVE : score/attnV products + folds (systems 0-3), softmax plumbing, evictions
  GPS : score/attnV products (systems 4-7)
  ACT : exp, qkv PSUM evictions
"""
import numpy as np

B, N, NS, G, E, H = 65536, 64, 8, 8, 16, 4
Dh = E // H
NCORES = 8
BC = B // NCORES          # batch per core
NT = BC // 128            # tiles per core

_F32 = np.float32


# ---------------------------------------------------------------- host folds
def _fold_constants(inp):
    W1 = np.asarray(inp["W1"], np.float64)
    b1 = np.asarray(inp["b1"], np.float64)
    W2 = np.asarray(inp["W2"], np.float64)
    b2 = np.asarray(inp["b2"], np.float64)
    Wqkv = np.asarray(inp["Wqkv_sys"], np.float64)
    bqkv = np.asarray(inp["bqkv_sys"], np.float64)
    Wo = np.asarray(inp["Wo_sys"], np.float64)
    bo = np.asarray(inp["bo_sys"], np.float64)
    Wsys = np.asarray(inp["Wsys"], np.float64)
    bsys = np.asarray(inp["bsys"], np.float64)
    Wqkv_x = np.asarray(inp["Wqkv_x"], np.float64)
    bqkv_x = np.asarray(inp["bqkv_x"], np.float64)
    Wo_x = np.asarray(inp["Wo_x"], np.float64)
    bo_x = np.asarray(inp["bo_x"], np.float64)
    sys_idx = np.asarray(inp["sys_idx"], np.int64)

    scale = 1.0 / np.sqrt(Dh)

    # R1: [64, 1024]; chunk c columns c*128..: (g,e) -> relu-pre h feats
    # (biomarker order follows sys_idx so that system s == h-chunk s)
    R1 = np.zeros((64, 1024))
    b1c = np.zeros((128, 8))
    for c in range(8):
        for g in range(8):
            n = int(sys_idx[c, g])
            R1[n, c * 128 + g * 16:(c * 128) + (g + 1) * 16] = W1[n]
            b1c[g * 16:(g + 1) * 16, c] = b1[n]

    # M2: h-chunk(s) -> qkv_s. cols (pt, d, q, h): pt*128 + d*32 + q*4 + h
    M2 = np.zeros((8, 128, 384))
    bias2 = np.zeros((8, 384))
    for s in range(8):
        for g in range(8):
            n = int(sys_idx[s, g])
            A = np.einsum('ef,je->fj', W2[n], Wqkv[s])     # [f16, j48]
            bj = b2[n] @ Wqkv[s].T + bqkv[s]
            A[:, :E] *= scale
            bj[:E] *= scale
            for pt in range(3):
                for h in range(H):
                    for d in range(Dh):
                        j = pt * E + h * Dh + d
                        col = pt * 128 + d * 32 + g * 4 + h
                        M2[s, g * 16:(g + 1) * 16, col] = A[:, j]
                        bias2[s, col] = bj[j]

    # C: o-chunk(s) rows (d', q, h) -> sys_emb cols (s-block, e')
    C = np.zeros((128, 128))
    biasC = np.zeros(128)
    for s in range(8):
        Cfull = np.zeros((128, 16))          # rows (g,(h,d)) orig order
        for g in range(8):
            Cfull[g * 16:(g + 1) * 16, :] = (Wsys[s][:, g * 16:(g + 1) * 16] @ Wo[s]).T
        for g in range(8):
            for h in range(H):
                for d in range(Dh):
                    C[d * 32 + g * 4 + h, s * 16:(s + 1) * 16] = Cfull[g * 16 + h * Dh + d]
        biasC[s * 16:(s + 1) * 16] = Wsys[s] @ np.tile(bo[s], G) + bsys[s]

    # Mx: all_sys rows (s,e) -> cross qkv cols (pt, d, pos, h)
    Mx = np.zeros((128, 384))
    biasx = np.zeros(384)
    Aq = Wqkv_x.T.copy()
    Aq[:, :E] *= scale
    bx = bqkv_x.copy()
    bx[:E] *= scale
    for s in range(8):
        for pt in range(3):
            for h in range(H):
                for d in range(Dh):
                    j = pt * E + h * Dh + d
                    col = pt * 128 + d * 32 + s * 4 + h
                    Mx[s * 16:(s + 1) * 16, col] = Aq[:, j]
                    biasx[col] = bx[j]

    # F: oc rows (d', pos, h) -> sf cols (pos, e')
    F = np.zeros((128, 128))
    biasF = np.zeros(128)
    for s in range(8):
        for h in range(H):
            for d in range(Dh):
                F[d * 32 + s * 4 + h, s * 16:(s + 1) * 16] = Wo_x.T[h * Dh + d]
        biasF[s * 16:(s + 1) * 16] = bo_x

    assert abs(b1).max() == 0, "nonzero b1 unsupported (merged relu path)"
    assert abs(bias2).max() == 0 and abs(biasC).max() == 0, "nonzero folded biases unsupported"
    assert abs(biasx).max() == 0 and abs(biasF).max() == 0, "nonzero folded biases unsupported"

    return dict(
        R1m=R1.astype(_F32), b1m=b1c.astype(_F32),
        M2m=np.concatenate([M2[s] for s in range(8)], axis=1).astype(_F32),   # [128, 3072]
        Cm=C.astype(_F32), Mxm=Mx.astype(_F32), Fm=F.astype(_F32),
        Im=np.eye(128, dtype=_F32),
    )


# ---------------------------------------------------------------- bass build
def _build_bass(ntiles):
    import concourse.bass as bass
    import concourse.bacc as bacc
    import concourse.mybir as mybir
    from concourse import tile
    from concourse.mybir import AluOpType as Op, ActivationFunctionType as Act

    f32 = mybir.dt.float32
    nc = bacc.Bacc(trn_type="TRN2")

    xd = nc.dram_tensor("xc", [ntiles * 128, 64], f32, kind="ExternalInput")
    R1d = nc.dram_tensor("R1m", [64, 1024], f32, kind="ExternalInput")
    b1d = nc.dram_tensor("b1m", [128, 8], f32, kind="ExternalInput")
    M2d = nc.dram_tensor("M2m", [128, 3072], f32, kind="ExternalInput")
    Cd = nc.dram_tensor("Cm", [128, 128], f32, kind="ExternalInput")
    Mxd = nc.dram_tensor("Mxm", [128, 384], f32, kind="ExternalInput")
    Fd = nc.dram_tensor("Fm", [128, 128], f32, kind="ExternalInput")
    Id = nc.dram_tensor("Im", [128, 128], f32, kind="ExternalInput")
    sfd = nc.dram_tensor("sf", [ntiles * 128, 128], f32, kind="ExternalOutput")
    asd = nc.dram_tensor("asys", [ntiles * 128, 128], f32, kind="ExternalOutput")

    bf = mybir.dt.bfloat16
    with tile.TileContext(nc) as tc:
        cp = tc.alloc_tile_pool(name="consts", bufs=1)
        R1s = cp.tile([64, 1024], f32)
        nc.sync.dma_start(R1s[:], R1d[:])
        b1s = cp.tile([128, 8], f32)
        nc.sync.dma_start(b1s[:], b1d[:])
        M2s = cp.tile([128, 3072], f32)
        nc.sync.dma_start(M2s[:], M2d[:])
        Cs = cp.tile([128, 128], f32)
        nc.sync.dma_start(Cs[:], Cd[:])
        Mxs = cp.tile([128, 384], f32)
        nc.sync.dma_start(Mxs[:], Mxd[:])
        Fs = cp.tile([128, 128], f32)
        nc.sync.dma_start(Fs[:], Fd[:])
        Is = cp.tile([128, 128], f32)
        nc.sync.dma_start(Is[:], Id[:])

        xp = tc.alloc_tile_pool(name="xin", bufs=3)
        pst = tc.alloc_tile_pool(name="pst", bufs=2, space="PSUM")
        psh = tc.alloc_tile_pool(name="psh", bufs=2, space="PSUM")
        psq = tc.alloc_tile_pool(name="psq", bufs=2, space="PSUM")
        psm = tc.alloc_tile_pool(name="psm", bufs=2, space="PSUM")
        hp = tc.alloc_tile_pool(name="hp", bufs=3)
        qkvp = tc.alloc_tile_pool(name="qkvp", bufs=3)
        bigp = tc.alloc_tile_pool(name="bigp", bufs=3)
        s2kp = tc.alloc_tile_pool(name="s2kp", bufs=3)
        denp = tc.alloc_tile_pool(name="denp", bufs=3)
        op_ = tc.alloc_tile_pool(name="op", bufs=2)
        tinyp = tc.alloc_tile_pool(name="tinyp", bufs=8)
        otp = tc.alloc_tile_pool(name="otp", bufs=2)
        outp = tc.alloc_tile_pool(name="outp", bufs=3)

        for t in range(ntiles):
            rows = slice(t * 128, (t + 1) * 128)
            # ---- x load + transpose
            xt = xp.tile([128, 64], f32, tag="xt")
            nc.sync.dma_start(xt[:], xd[rows, :])
            xT_ps = pst.tile([64, 128], f32, tag="tps")
            nc.tensor.transpose(xT_ps[:], xt[:], Is[:])
            xTs = xp.tile([64, 128], f32, tag="xTs")
            nc.vector.tensor_copy(xTs[:], xT_ps[:])

            # ---- h chunks (feature-major, fp32); 4 chunks share a PSUM bank
            # (b1 == 0 asserted at fold time, so relu needs no per-chunk bias)
            hgrp = []
            for g2 in range(2):
                hps = psh.tile([128, 512], f32, tag="hps")
                for c4 in range(4):
                    c = g2 * 4 + c4
                    nc.tensor.matmul(hps[:, c4 * 128:(c4 + 1) * 128],
                                     R1s[:, c * 128:(c + 1) * 128], xTs[:])
                hg = hp.tile([128, 512], f32, tag=f"hg{g2}")
                nc.vector.tensor_scalar(hg[:], hps[:], 0.0, None, Op.max)
                hgrp.append(hg)
            hs = [hgrp[c // 4][:, (c % 4) * 128:(c % 4 + 1) * 128] for c in range(8)]

            # ---- qkv (batch-major; evicted to bf16 by ACT)
            # QKV layout (pt,s,d,q,h): addr = pt*1024 + s*128 + d*32 + q*4 + h
            QKV = qkvp.tile([128, 3072], bf, tag="qkv")
            QKVv = QKV[:].rearrange("p (pt s d q h) -> p pt s d q h", pt=3, s=8, d=4, q=8, h=4)
            for s in range(8):
                qps = psq.tile([128, 384], f32, tag="qps")
                nc.tensor.matmul(qps[:], hs[s], M2s[:, s * 384:(s + 1) * 384])
                dst = QKVv[:, :, s]
                src = qps[:].rearrange("p (pt d q h) -> p pt d q h", pt=3, d=4, q=8, h=4)
                nc.scalar.copy(dst, src)

            Qv = QKVv[:, 0]    # [128, s8, d4, q8, h4]
            Kv = QKVv[:, 1]
            Vv = QKVv[:, 2]
            Qb = Qv.unsqueeze(4).broadcast_to([128, 8, 4, 8, 8, 4])   # (s,d,q,k0,h)
            Kb = Kv.unsqueeze(3).broadcast_to([128, 8, 4, 8, 8, 4])   # (s,d,q0,k,h)
            Vb = Vv.unsqueeze(3).broadcast_to([128, 8, 4, 8, 8, 4])

            # ---- system attention (bf16 core; exp on ACT)
            # T layout (s,d,q,k,h); systems 0-3 on DVE, 4-7 on GPSIMD
            T = bigp.tile([128, 8192], bf, tag="big")
            T6 = T[:].rearrange("p (s d q k h) -> p s d q k h", s=8, d=4, q=8, k=8, h=4)
            nc.vector.tensor_tensor(T6[:, 0:5], Qb[:, 0:5], Kb[:, 0:5], Op.mult)
            nc.gpsimd.tensor_tensor(T6[:, 5:8], Qb[:, 5:8], Kb[:, 5:8], Op.mult)
            S2 = s2kp.tile([128, 4096], bf, tag="s4k")
            S2v = S2[:].rearrange("p (s d q k h) -> p s d q k h", s=8, d=2, q=8, k=8, h=4)
            nc.vector.tensor_tensor(S2v[:], T6[:, :, 0:2], T6[:, :, 2:4], Op.add)
            S = s2kp.tile([128, 2048], bf, tag="s2k")
            Sv = S[:].rearrange("p (s q k h) -> p s q k h", s=8, q=8, k=8, h=4)
            nc.vector.tensor_tensor(Sv[:].unsqueeze(2), S2v[:, :, 0:1], S2v[:, :, 1:2], Op.add)
            Ee = s2kp.tile([128, 2048], bf, tag="s2k")
            nc.scalar.activation(Ee[:, 0:1024], S[:, 0:1024], Act.Exp)
            nc.scalar.activation(Ee[:, 1024:2048], S[:, 1024:2048], Act.Exp)
            # denominators: fold over k; layout (s,q,k,h)
            Ev = Ee[:].rearrange("p (sq k h) -> p sq k h", sq=64, k=8, h=4)
            D2 = denp.tile([128, 64, 4, 4], f32, tag="d2")
            nc.vector.tensor_tensor(D2[:], Ev[:, :, 0:4], Ev[:, :, 4:8], Op.add)
            D4 = denp.tile([128, 64, 2, 4], f32, tag="d4")
            nc.vector.tensor_tensor(D4[:], D2[:, :, 0:2], D2[:, :, 2:4], Op.add)
            Dd = denp.tile([128, 64, 1, 4], f32, tag="dd")
            nc.vector.tensor_tensor(Dd[:], D4[:, :, 0:1], D4[:, :, 1:2], Op.add)
            Rv = denp.tile([128, 256], f32, tag="rv")
            nc.vector.reciprocal(Rv[:], Dd[:].rearrange("p a b c -> p (a b c)"))
            # attn @ V: P layout (s,d,q,k,h)
            Eb = Ee[:].rearrange("p (s q k h) -> p s q k h", s=8, q=8, k=8, h=4)
            Eb = Eb.unsqueeze(2).broadcast_to([128, 8, 4, 8, 8, 4])   # (s,d0,q,k,h)
            P = bigp.tile([128, 8192], bf, tag="big")
            P6 = P[:].rearrange("p (s d q k h) -> p s d q k h", s=8, d=4, q=8, k=8, h=4)
            nc.vector.tensor_tensor(P6[:, 0:5], Eb[:, 0:5], Vb[:, 0:5], Op.mult)
            nc.gpsimd.tensor_tensor(P6[:, 5:8], Eb[:, 5:8], Vb[:, 5:8], Op.mult)
            Pv = P[:].rearrange("p (sdq k h) -> p sdq k h", sdq=256, k=8, h=4)
            K1 = s2kp.tile([128, 256, 4, 4], bf, tag="s4k")
            nc.vector.tensor_tensor(K1[:, 0:128], Pv[:, 0:128, 0:4], Pv[:, 0:128, 4:8], Op.add)
            nc.vector.tensor_tensor(K1[:, 128:256], Pv[:, 128:256, 0:4], Pv[:, 128:256, 4:8], Op.add)
            K2 = s2kp.tile([128, 256, 2, 4], bf, tag="s2k")
            nc.vector.tensor_tensor(K2[:], K1[:, :, 0:2], K1[:, :, 2:4], Op.add)
            num = op_.tile([128, 256, 1, 4], bf, tag="num")
            nc.vector.tensor_tensor(num[:], K2[:, :, 0:1], K2[:, :, 1:2], Op.add)
            o = op_.tile([128, 1024], f32, tag="o")
            # o layout (s, d, q, h): system chunks are contiguous 128-col slices
            ov = o[:].rearrange("p (s d q h) -> p s d q h", s=8, d=4, q=8, h=4)
            nc.vector.tensor_tensor(
                ov[:],
                num[:].rearrange("p a b c -> p (a b c)").rearrange("p (s d q h) -> p s d q h", s=8, d=4, q=8, h=4),
                Rv[:].rearrange("p (s q h) -> p s q h", s=8, q=8, h=4).unsqueeze(2).broadcast_to([128, 8, 4, 8, 4]),
                Op.mult)

            # ---- o -> all_sys (4 transposes share a PSUM bank; 2 evicts)
            as_ps = psm.tile([128, 128], f32, tag="mps")
            oTg = []
            for g2 in range(2):
                oT_ps = pst.tile([128, 512], f32, tag="tps")
                for s4 in range(4):
                    s = g2 * 4 + s4
                    nc.tensor.transpose(oT_ps[:, s4 * 128:(s4 + 1) * 128],
                                        o[:, s * 128:(s + 1) * 128], Is[:])
                og = otp.tile([128, 512], f32, tag=f"oTg{g2}")
                nc.vector.tensor_copy(og[:], oT_ps[:])
                oTg.append(og)
            for s in range(8):
                nc.tensor.matmul(as_ps[:, s * 16:(s + 1) * 16],
                                 oTg[s // 4][:, (s % 4) * 128:(s % 4 + 1) * 128],
                                 Cs[:, s * 16:(s + 1) * 16])
            asys_s = outp.tile([128, 128], f32, tag="asys")
            nc.vector.tensor_copy(asys_s[:], as_ps[:])
            nc.sync.dma_start(asd[rows, :], asys_s[:])

            # ---- cross qkv
            asT_ps = pst.tile([128, 128], f32, tag="tps")
            nc.tensor.transpose(asT_ps[:], asys_s[:], Is[:])
            asTs = otp.tile([128, 128], f32, tag="asTs")
            nc.vector.tensor_copy(asTs[:], asT_ps[:])
            cq_ps = psq.tile([128, 384], f32, tag="qps")
            nc.tensor.matmul(cq_ps[:], asTs[:], Mxs[:])
            QKVc = qkvp.tile([128, 384], bf, tag="qkvc")
            nc.vector.tensor_copy(QKVc[:], cq_ps[:])
            QKVcv = QKVc[:].rearrange("p (pt d s h) -> p pt d s h", pt=3, d=4, s=8, h=4)
            Qcb = QKVcv[:, 0].unsqueeze(3).broadcast_to([128, 4, 8, 8, 4])
            Kcb = QKVcv[:, 1].unsqueeze(2).broadcast_to([128, 4, 8, 8, 4])
            Vcb = QKVcv[:, 2].unsqueeze(2).broadcast_to([128, 4, 8, 8, 4])

            # ---- cross attention (bf16 core)
            Tc = op_.tile([128, 1024], bf, tag="tc")
            Tc5 = Tc[:].rearrange("p (d q k h) -> p d q k h", d=4, q=8, k=8, h=4)
            nc.vector.tensor_tensor(Tc5, Qcb, Kcb, Op.mult)
            Sc2 = tinyp.tile([128, 512], bf, tag="tiny")
            nc.vector.tensor_tensor(Sc2[:], Tc[:, 0:512], Tc[:, 512:1024], Op.add)
            Sc = tinyp.tile([128, 256], bf, tag="tiny")
            nc.vector.tensor_tensor(Sc[:], Sc2[:, 0:256], Sc2[:, 256:512], Op.add)
            Ec = tinyp.tile([128, 256], bf, tag="tiny")
            nc.scalar.activation(Ec[:], Sc[:], Act.Exp)
            Ecv = Ec[:].rearrange("p (q k h) -> p q k h", q=8, k=8, h=4)
            Dc2 = tinyp.tile([128, 8, 4, 4], f32, tag="tiny")
            nc.vector.tensor_tensor(Dc2[:], Ecv[:, :, 0:4], Ecv[:, :, 4:8], Op.add)
            Dc4 = tinyp.tile([128, 8, 2, 4], f32, tag="tiny")
            nc.vector.tensor_tensor(Dc4[:], Dc2[:, :, 0:2], Dc2[:, :, 2:4], Op.add)
            Dc = tinyp.tile([128, 8, 1, 4], f32, tag="tiny")
            nc.vector.tensor_tensor(Dc[:], Dc4[:, :, 0:1], Dc4[:, :, 1:2], Op.add)
            Rc = tinyp.tile([128, 32], f32, tag="tinyf")
            nc.vector.reciprocal(Rc[:], Dc[:].rearrange("p a b c -> p (a b c)"))
            Ecb = Ecv.unsqueeze(1).broadcast_to([128, 4, 8, 8, 4])
            Pc = op_.tile([128, 1024], bf, tag="pc")
            Pc5 = Pc[:].rearrange("p (d q k h) -> p d q k h", d=4, q=8, k=8, h=4)
            nc.vector.tensor_tensor(Pc5, Ecb, Vcb, Op.mult)
            Pcv = Pc[:].rearrange("p (dq k h) -> p dq k h", dq=32, k=8, h=4)
            Kc1 = tinyp.tile([128, 32, 4, 4], bf, tag="tiny")
            nc.vector.tensor_tensor(Kc1[:], Pcv[:, :, 0:4], Pcv[:, :, 4:8], Op.add)
            Kc2 = tinyp.tile([128, 32, 2, 4], bf, tag="tiny")
            nc.vector.tensor_tensor(Kc2[:], Kc1[:, :, 0:2], Kc1[:, :, 2:4], Op.add)
            numc = tinyp.tile([128, 32, 1, 4], bf, tag="tiny")
            nc.vector.tensor_tensor(numc[:], Kc2[:, :, 0:1], Kc2[:, :, 1:2], Op.add)
            oc = tinyp.tile([128, 128], f32, tag="tinyf")
            nc.vector.tensor_tensor(
                oc[:].rearrange("p (dp r) -> p dp r", dp=4, r=32),
                numc[:].rearrange("p a b c -> p (a b c)").rearrange("p (dp r) -> p dp r", dp=4, r=32),
                Rc[:].unsqueeze(1).broadcast_to([128, 4, 32]), Op.mult)

            # ---- oc -> system_features
            ocT_ps = pst.tile([128, 128], f32, tag="tps")
            nc.tensor.transpose(ocT_ps[:], oc[:], Is[:])
            ocTs = otp.tile([128, 128], f32, tag="ocTs")
            nc.vector.tensor_copy(ocTs[:], ocT_ps[:])
            sf_ps = psm.tile([128, 128], f32, tag="mps")
            nc.tensor.matmul(sf_ps[:], ocTs[:], Fs[:])
            sf_s = outp.tile([128, 128], f32, tag="sf")
            nc.vector.tensor_copy(sf_s[:], sf_ps[:])
            nc.sync.dma_start(sfd[rows, :], sf_s[:])

        for pool in (outp, otp, tinyp, op_, denp, s2kp, bigp, qkvp, hp, psm, psq, psh, pst, xp, cp):
            pool.release()

    nc.finalize()
    return nc


_CACHE = {}
LAST_RESULT = None


def _get_nc(ntiles):
    if ntiles not in _CACHE:
        _CACHE[ntiles] = _build_bass(ntiles)
    return _CACHE[ntiles]


# ---------------------------------------------------------------- entry point
def kernel(**inputs):
    import os
    os.environ.setdefault("BASS_NEVER_TRACE", "1")  # no NTFF hook in this axon client
    from concourse.bass_utils import run_bass_kernel_spmd

    x = np.ascontiguousarray(np.asarray(inputs["x"], _F32))
    Bn = x.shape[0]
    consts = _fold_constants(inputs)

    ncores = NCORES
    bc = Bn // ncores
    ntiles = bc // 128
    nc = _get_nc(ntiles)

    in_maps = []
    for c in range(ncores):
        m = {"xc": x[c * bc:(c + 1) * bc]}
        m.update(consts)
        in_maps.append(m)

    res = run_bass_kernel_spmd(nc, in_maps, core_ids=list(range(ncores)))
    global LAST_RESULT
    LAST_RESULT = res
    sf = np.concatenate([res.results[c]["sf"] for c in range(ncores)], axis=0)
    asys = np.concatenate([res.results[c]["asys"] for c in range(ncores)], axis=0)
    return sf, asys.reshape(Bn, NS, E)
